# revision 1
# baseline (speedup 1.0000x reference)
"""Trainium2 Bass kernel for nn_CrossAttentionLayer.

Reference computation (per batch element b):
    q = x @ Wq            [N, INNER]   (heads: INNER = H*Dh)
    k = ctx @ Wk          [J, INNER]
    v = ctx @ Wv          [J, INNER]
    sim = q_h @ k_h.T * scale   per head -> softmax over J -> @ v_h
    out = concat_heads @ Wo + bo

Sharding: batch (B=8) across 8 cores, one batch element per core, weights
replicated.  No collectives needed.

Per-core plan (all matmuls bf16 operands, fp32 PSUM accumulation):
  - transpose x -> xT [QD, N], ctx -> ctxT [CD, J] via PE transposes
  - QT [INNER, N] = Wq.T @ xT   (stationary Wq chunks, moving xT)
  - KT [INNER, J] = Wk.T @ ctxT
  - V  [J, INNER] = ctxT.T @ Wv, stored padded per head with a ones column
  - per head h: S^T [J, N] = KT_h.T-contract-d @ QT_h  (K=64 contraction)
      P^T = exp(scale * S^T)  on ACT, written bf16
      O[n, 64+1] = sum_jc P^T_jc.T @ Vpad_h_jc   (ones col -> softmax denom)
      evict O unnormalized (ACT), denom col to den buffer
  - normalize per n-tile: rden = 1/den (DVE), O *= rden (broadcast mul)
  - transpose O -> OT [INNER, N]; out = OT.T @ Wo + bo -> DMA out
"""

import sys

if "/opt/trn_rl_repo" not in sys.path:
    sys.path.insert(0, "/opt/trn_rl_repo")

import numpy as np

import concourse.bass as bass
import concourse.mybir as mybir
import concourse.bacc as bacc
import concourse.tile as tile
from concourse import bass_utils
from concourse.masks import make_identity

P = 128
B, N, J = 8, 2048, 1024
QD, CD, H, Dh = 1024, 768, 16, 64
INNER = H * Dh
NT = N // P      # 16 n tiles
JC = J // P      # 8 context chunks
QC = QD // P     # 8 x-feature chunks
CC = CD // P     # 6 ctx-feature chunks
IC = INNER // P  # 8 inner chunks
NBW = 512        # moving-operand block width
NB = N // NBW    # 4
SCALE = float(Dh) ** -0.5

F32 = mybir.dt.float32
BF16 = mybir.dt.bfloat16
EXP = mybir.ActivationFunctionType.Exp

_CACHE = {}


def _build_module():
    nc = bacc.Bacc("TRN2", target_bir_lowering=False, debug=False)

    x_d = nc.dram_tensor("x", [N, QD], F32, kind="ExternalInput")
    ctx_d = nc.dram_tensor("context", [J, CD], F32, kind="ExternalInput")
    wq_d = nc.dram_tensor("Wq", [QD, INNER], F32, kind="ExternalInput")
    wk_d = nc.dram_tensor("Wk", [CD, INNER], F32, kind="ExternalInput")
    wv_d = nc.dram_tensor("Wv", [CD, INNER], F32, kind="ExternalInput")
    wo_d = nc.dram_tensor("Wo", [INNER, QD], F32, kind="ExternalInput")
    bo_d = nc.dram_tensor("bo", [QD], F32, kind="ExternalInput")
    out_d = nc.dram_tensor("out", [N, QD], F32, kind="ExternalOutput")

    with tile.TileContext(nc) as tc:
        _emit(nc, tc, x_d, ctx_d, wq_d, wk_d, wv_d, wo_d, bo_d, out_d)

    nc.compile()
    return nc


def _emit(nc, tc, x_d, ctx_d, wq_d, wk_d, wv_d, wo_d, bo_d, out_d):
    from contextlib import ExitStack

    est = ExitStack()
    with est:
        # ---------- constants ----------
        const = est.enter_context(tc.tile_pool(name="const", bufs=1))
        ones_row = const.tile([1, P], F32, name="ones_row")
        nc.vector.memset(ones_row[:], 1.0)
        ones_bf = const.tile([1, P], BF16, name="ones_bf")
        nc.vector.memset(ones_bf[:], 1.0)
        bo_sb = const.tile([1, QD], F32, name="bo_sb")
        nc.sync.dma_start(bo_sb[:], bo_d[:].unsqueeze(0))
        bias_bc = const.tile([P, QD], BF16, name="bias_bc")

        with tc.tile_pool(name="cpsum", bufs=2, space="PSUM") as cpsum:
            for qb in range(QD // NBW):
                bp = cpsum.tile([P, NBW], F32, name="bp", tag="bp")
                nc.tensor.matmul(
                    bp[:], ones_row[:, :], bo_sb[:, qb * NBW:(qb + 1) * NBW],
                    start=True, stop=True,
                )
                nc.vector.tensor_copy(bias_bc[:, qb * NBW:(qb + 1) * NBW], bp[:])

        # ---------- bf16 casts staged in DRAM (flat = 1 descriptor each) ----
        dram = est.enter_context(tc.tile_pool(name="dram", bufs=1, space="DRAM"))
        x_bf = dram.tile([N, QD], BF16, name="x_bf")
        ctx_bf = dram.tile([J, CD], BF16, name="ctx_bf")
        wq_bf = dram.tile([QD, INNER], BF16, name="wq_bf")
        wk_bf = dram.tile([CD, INNER], BF16, name="wk_bf")
        wv_bf = dram.tile([CD, INNER], BF16, name="wv_bf")
        wo_bf = dram.tile([INNER, QD], BF16, name="wo_bf")
        nc.gpsimd.dma_start(ctx_bf[:].flatten(), ctx_d[:].flatten())
        nc.gpsimd.dma_start(x_bf[:].flatten(), x_d[:].flatten())
        nc.gpsimd.dma_start(wk_bf[:].flatten(), wk_d[:].flatten())
        nc.gpsimd.dma_start(wv_bf[:].flatten(), wv_d[:].flatten())
        nc.gpsimd.dma_start(wq_bf[:].flatten(), wq_d[:].flatten())
        nc.gpsimd.dma_start(wo_bf[:].flatten(), wo_d[:].flatten())

        # ---------- persistent activations ----------
        qkv = est.enter_context(tc.tile_pool(name="qkv", bufs=1))
        qt = [qkv.tile([P, N], BF16, name=f"qt{c}", tag=f"qt{c}") for c in range(IC)]
        kt = [qkv.tile([P, J], BF16, name=f"kt{c}", tag=f"kt{c}") for c in range(IC)]
        # v padded: per head 64 cols of V then a ones column (65 per head)
        vp = [qkv.tile([P, H * 65], BF16, name=f"vp{c}", tag=f"vp{c}")
              for c in range(JC)]

        o_bf = est.enter_context(tc.tile_pool(name="o_bf", bufs=1))
        # ot[ic] [P, N]: normalized attention output, transposed layout
        # [INNER, N]; head h lives in chunk h//2, partitions (h%2)*64.
        ot = [o_bf.tile([P, N], BF16, name=f"ot{c}", tag=f"ot{c}")
              for c in range(IC)]

        wo_pool = est.enter_context(tc.tile_pool(name="wo_pool", bufs=1))
        wo_sb = wo_pool.tile([P, IC * QD], BF16, name="wo_sb")
        nc.sync.dma_start(
            wo_sb[:].rearrange("p (c n) -> p c n", c=IC),
            wo_bf[:].rearrange("(c p) n -> p c n", p=P))

        # ---------- phase A: ctx path (wk, wv, ctxT, KT, Vpad) ----------
        with ExitStack() as actx:
            wkv = actx.enter_context(tc.tile_pool(name="wkv", bufs=1))
            wk_sb = wkv.tile([P, CC * INNER], BF16, name="wk_sb")
            wv_sb = wkv.tile([P, CC * INNER], BF16, name="wv_sb")
            nc.sync.dma_start(
                wk_sb[:].rearrange("p (c n) -> p c n", c=CC),
                wk_bf[:].rearrange("(c p) n -> p c n", p=P))
            nc.sync.dma_start(
                wv_sb[:].rearrange("p (c n) -> p c n", c=CC),
                wv_bf[:].rearrange("(c p) n -> p c n", p=P))

            ctxT_p = actx.enter_context(tc.tile_pool(name="ctxT_p", bufs=1))
            ctxT = [ctxT_p.tile([P, J], BF16, name=f"ctxT{c}", tag=f"ctxT{c}")
                    for c in range(CC)]

            ppsum = actx.enter_context(
                tc.tile_pool(name="ppsum", bufs=4, space="PSUM"))

            for cc in range(CC):
                nc.sync.dma_start(
                    ctxT[cc][:], ctx_bf[:, cc * P:(cc + 1) * P], transpose=True)

            # KT[ic] [P, J]: stationary Wk chunk, moving ctxT
            for ic in range(IC):
                for jb in range(J // NBW):
                    kp = ppsum.tile([P, NBW], F32, name="kp", tag="pp")
                    for cc in range(CC):
                        nc.tensor.matmul(
                            kp[:],
                            wk_sb[:, cc * INNER + ic * P: cc * INNER + (ic + 1) * P],
                            ctxT[cc][:, jb * NBW:(jb + 1) * NBW],
                            start=(cc == 0), stop=(cc == CC - 1),
                        )
                    nc.vector.tensor_copy(
                        kt[ic][:, jb * NBW:(jb + 1) * NBW], kp[:])

            # V natural [J, INNER] -> padded per head (65 cols per head)
            for jc in range(JC):
                for vb in range(INNER // NBW):
                    vpp = ppsum.tile([P, NBW], F32, name="vpp", tag="pp")
                    for cc in range(CC):
                        nc.tensor.matmul(
                            vpp[:],
                            ctxT[cc][:, jc * P:(jc + 1) * P],
                            wv_sb[:, cc * INNER + vb * NBW: cc * INNER + (vb + 1) * NBW],
                            start=(cc == 0), stop=(cc == CC - 1),
                        )
                    hpb = NBW // Dh  # heads per block = 8
                    dst = vp[jc][:, vb * hpb * 65:(vb + 1) * hpb * 65]
                    dst = dst.rearrange("p (h e) -> p h e", e=65)[:, :, 0:64]
                    src = vpp[:].rearrange("p (h e) -> p h e", e=Dh)
                    nc.vector.tensor_copy(dst, src)
                ones_cols = vp[jc][:].rearrange(
                    "p (h e) -> p h e", e=65)[:, :, 64:65]
                nc.vector.memset(ones_cols, 1.0)

        # ---------- phase B: x path (wq, xT, QT) ----------
        with ExitStack() as bctx:
            wq_pool = bctx.enter_context(tc.tile_pool(name="wq_pool", bufs=1))
            wq_sb = wq_pool.tile([P, QC * INNER], BF16, name="wq_sb")
            nc.sync.dma_start(
                wq_sb[:].rearrange("p (c n) -> p c n", c=QC),
                wq_bf[:].rearrange("(c p) n -> p c n", p=P))

            xT_p = bctx.enter_context(tc.tile_pool(name="xT_p", bufs=1))
            xT = [xT_p.tile([P, N], BF16, name=f"xT{c}", tag=f"xT{c}")
                  for c in range(QC)]

            ppsum2 = bctx.enter_context(
                tc.tile_pool(name="ppsum2", bufs=4, space="PSUM"))

            for qc in range(QC):
                nc.sync.dma_start(
                    xT[qc][:], x_bf[:, qc * P:(qc + 1) * P], transpose=True)

            for ic in range(IC):
                for nb in range(NB):
                    qp = ppsum2.tile([P, NBW], F32, name="qp", tag="qp2")
                    for qc in range(QC):
                        nc.tensor.matmul(
                            qp[:],
                            wq_sb[:, qc * INNER + ic * P: qc * INNER + (ic + 1) * P],
                            xT[qc][:, nb * NBW:(nb + 1) * NBW],
                            start=(qc == 0), stop=(qc == QC - 1),
                        )
                    nc.vector.tensor_copy(
                        qt[ic][:, nb * NBW:(nb + 1) * NBW], qp[:])

        # ---------- phase C: attention ----------
        # PV in O^T orientation: stationary Vpad [128, 65], moving P^T
        # [128, 512] -> psum [65, 512]; row 64 is the softmax denominator.
        # Normalize: denom row -> den_all (ACT), PE ones-broadcast to 64
        # partitions, DVE reciprocal, DVE multiply psum*recip -> ot (bf16).
        with ExitStack() as cctx:
            pt_pool = cctx.enter_context(tc.tile_pool(name="pt_pool", bufs=2))
            rd_pool = cctx.enter_context(tc.tile_pool(name="rd_pool", bufs=2))
            den_pool = cctx.enter_context(tc.tile_pool(name="den_pool", bufs=4))
            spsum = cctx.enter_context(
                tc.tile_pool(name="spsum", bufs=2, space="PSUM"))
            pvpsum = cctx.enter_context(
                tc.tile_pool(name="pvpsum", bufs=2, space="PSUM"))
            bcpsum = cctx.enter_context(
                tc.tile_pool(name="bcpsum", bufs=2, space="PSUM"))

            for h in range(H):
                ic = h // 2
                po = (h % 2) * Dh
                pts = []
                for jc in range(JC):
                    ptile = pt_pool.tile([P, N], BF16, name=f"pt{jc}",
                                         tag=f"pt{jc}")
                    pts.append(ptile)
                    for half in range(2):
                        sp = spsum.tile([P, 2 * NBW], F32, name="sp", tag="sp")
                        for nbh in range(2):
                            nb = half * 2 + nbh
                            nc.tensor.matmul(
                                sp[:, nbh * NBW:(nbh + 1) * NBW],
                                kt[ic][po:po + Dh, jc * P:(jc + 1) * P],
                                qt[ic][po:po + Dh, nb * NBW:(nb + 1) * NBW],
                                start=True, stop=True,
                            )
                        nc.scalar.activation(
                            ptile[:, half * 2 * NBW:(half + 1) * 2 * NBW],
                            sp[:], EXP, scale=SCALE)

                for nb in range(NB):
                    pv = pvpsum.tile([65, NBW], F32, name="pv", tag="pv")
                    for jc in range(JC):
                        nc.tensor.matmul(
                            pv[:],
                            vp[jc][:, h * 65: h * 65 + 65],
                            pts[jc][:, nb * NBW:(nb + 1) * NBW],
                            start=(jc == 0), stop=(jc == JC - 1),
                        )
                    # denominator row -> bf16 (ACT), broadcast to 64
                    # partitions via K=1 matmul, reciprocal, multiply.
                    den_t = den_pool.tile([1, NBW], BF16, name="den_t",
                                          tag="den_t")
                    nc.vector.tensor_copy(den_t[:], pv[64:65, :])
                    bc = bcpsum.tile([Dh, NBW], F32, name="bc", tag="bc")
                    nc.tensor.matmul(
                        bc[:], ones_bf[:, 0:Dh], den_t[:],
                        start=True, stop=True)
                    rden = rd_pool.tile([Dh, NBW], F32, name="rden", tag="rden")
                    nc.vector.reciprocal(rden[:], bc[:])
                    nc.vector.tensor_tensor(
                        ot[ic][po:po + Dh, nb * NBW:(nb + 1) * NBW],
                        pv[0:Dh, :], rden[:], op=mybir.AluOpType.mult)

        # ---------- phase D: output projection ----------
        with ExitStack() as dctx:
            opsum = dctx.enter_context(
                tc.tile_pool(name="opsum", bufs=4, space="PSUM"))
            ostage_p = dctx.enter_context(tc.tile_pool(name="ostage_p", bufs=4))

            for nt in range(NT):
                for qb in range(QD // NBW):
                    op = opsum.tile([P, NBW], F32, name="op", tag="op")
                    for ic in range(IC):
                        nc.tensor.matmul(
                            op[:],
                            ot[ic][:, nt * P:(nt + 1) * P],
                            wo_sb[:, ic * QD + qb * NBW: ic * QD + (qb + 1) * NBW],
                            start=(ic == 0), stop=(ic == IC - 1),
                        )
                    ostage = ostage_p.tile([P, NBW], F32, name="ostage",
                                           tag="ostage")
                    nc.vector.tensor_tensor(
                        ostage[:], op[:], bias_bc[:, qb * NBW:(qb + 1) * NBW],
                        op=mybir.AluOpType.add)
                    nc.sync.dma_start(
                        out_d[nt * P:(nt + 1) * P, qb * NBW:(qb + 1) * NBW],
                        ostage[:])


def _get_module():
    if "nc" not in _CACHE:
        _CACHE["nc"] = _build_module()
    return _CACHE["nc"]


def kernel(x, context, Wq, Wk, Wv, Wo, bo):
    nc = _get_module()
    x = np.asarray(x, dtype=np.float32)
    context = np.asarray(context, dtype=np.float32)
    Wq = np.asarray(Wq, dtype=np.float32)
    Wk = np.asarray(Wk, dtype=np.float32)
    Wv = np.asarray(Wv, dtype=np.float32)
    Wo = np.asarray(Wo, dtype=np.float32)
    bo = np.asarray(bo, dtype=np.float32)

    in_maps = [
        {
            "x": np.ascontiguousarray(x[b]),
            "context": np.ascontiguousarray(context[b]),
            "Wq": Wq, "Wk": Wk, "Wv": Wv, "Wo": Wo, "bo": bo,
        }
        for b in range(B)
    ]
    res = bass_utils.run_bass_kernel_spmd(nc, in_maps, core_ids=list(range(B)))
    return np.stack([res.results[b]["out"] for b in range(B)], axis=0)


if __name__ == "__main__":
    nc = _get_module()
    print("module built and compiled OK")



# revision 27
# speedup vs baseline: 1.3847x; 1.3847x over previous
"""Trainium2 Bass kernel for nn_CrossAttentionLayer (v3).

Reference computation (per batch element b):
    q = x @ Wq            [N, INNER]   (heads: INNER = H*Dh)
    k = ctx @ Wk          [J, INNER]
    v = ctx @ Wv          [J, INNER]
    sim = q_h @ k_h.T * scale   per head -> softmax over J -> @ v_h
    out = concat_heads @ Wo + bo

Sharding: batch (B=8) across 8 cores, one batch element per core, weights
replicated.  No collectives.

v3 design:
  - No DRAM->DRAM bf16 staging: f32 inputs stream through a small SBUF
    ring; ctx/x are transposed on the (otherwise idle) tensor engine at
    the front, weights are cast f32->bf16 on the (idle) gpsimd engine.
    Wq/Wk load per-ic so the first attention head starts ~40us in.
  - PV computed in natural-O orientation: stationary = P^T tile [128 j,
    128 n] (full contraction AND full output partitions), moving = V
    padded with a per-head ones column -> softmax denominator lands in
    PSUM column 64 of each slice for free.
  - Normalize with DVE reciprocal (batched, strided view) + tensor_scalar
    per-partition multiply; PE-transpose the normalized O tiles into O^T
    for the output projection.
  - N split in two halves: exp runs at [128, 1024] grain on ACT (its
    efficient grain) and the half-0 output projection overlaps half-1
    attention.  Q-projection for half 1, x-half-1 transposes and out-proj
    chunks are woven into the head loops so PE stays busy while ACT chews
    the exp stream.
"""

import sys

if "/opt/trn_rl_repo" not in sys.path:
    sys.path.insert(0, "/opt/trn_rl_repo")

from contextlib import ExitStack

import numpy as np

import concourse.bass as bass
import concourse.mybir as mybir
import concourse.bacc as bacc
import concourse.tile as tile
from concourse import bass_utils
from concourse.masks import make_identity

P = 128
B, N, J = 8, 2048, 1024
QD, CD, H, Dh = 1024, 768, 16, 64
INNER = H * Dh
NT = N // P      # 16 n tiles
JC = J // P      # 8 context chunks
QC = QD // P     # 8 x-feature chunks
CC = CD // P     # 6 ctx-feature chunks
IC = INNER // P  # 8 inner chunks
NH = N // 2      # 1024: half of the n dimension
NTH = NH // P    # 8 n tiles per half
NBW = 512        # psum matmul block width
SCALE = float(Dh) ** -0.5

F32 = mybir.dt.float32
BF16 = mybir.dt.bfloat16
EXP = mybir.ActivationFunctionType.Exp
MUL = mybir.AluOpType.mult
ADD = mybir.AluOpType.add

_CACHE = {}


def _build_module():
    nc = bacc.Bacc("TRN2", target_bir_lowering=False, debug=False)

    x_d = nc.dram_tensor("x", [N, QD], F32, kind="ExternalInput")
    ctx_d = nc.dram_tensor("context", [J, CD], F32, kind="ExternalInput")
    wq_d = nc.dram_tensor("Wq", [QD, INNER], F32, kind="ExternalInput")
    wk_d = nc.dram_tensor("Wk", [CD, INNER], F32, kind="ExternalInput")
    wv_d = nc.dram_tensor("Wv", [CD, INNER], F32, kind="ExternalInput")
    wo_d = nc.dram_tensor("Wo", [INNER, QD], F32, kind="ExternalInput")
    bo_d = nc.dram_tensor("bo", [QD], F32, kind="ExternalInput")
    out_d = nc.dram_tensor("out", [N, QD], F32, kind="ExternalOutput")

    with tile.TileContext(nc) as tc:
        _emit(nc, tc, x_d, ctx_d, wq_d, wk_d, wv_d, wo_d, bo_d, out_d)

    nc.compile()
    return nc


def _emit(nc, tc, x_d, ctx_d, wq_d, wk_d, wv_d, wo_d, bo_d, out_d):
    est = ExitStack()
    with est:
        # ---------- constants ----------
        const = est.enter_context(tc.tile_pool(name="const", bufs=1))
        ident = const.tile([P, P], BF16, name="ident")
        make_identity(nc, ident[:])
        ident32 = const.tile([P, P], F32, name="ident32")
        make_identity(nc, ident32[:])
        ones_row = const.tile([1, P], BF16, name="ones_row")
        nc.vector.memset(ones_row[:], 1.0)
        bo_sb = const.tile([1, QD], BF16, name="bo_sb")
        nc.gpsimd.dma_start(bo_sb[:], bo_d[:].unsqueeze(0))
        bias_bc = const.tile([P, QD], BF16, name="bias_bc")

        # ---------- persistent activations ----------
        persist = est.enter_context(tc.tile_pool(name="persist", bufs=1))
        qt = [persist.tile([P, N], BF16, name=f"qt{c}") for c in range(IC)]
        kt = [persist.tile([P, J], BF16, name=f"kt{c}") for c in range(IC)]
        # v natural per jc, padded per head: 64 v columns + a ones column
        vp = [persist.tile([P, H * 65], BF16, name=f"vp{c}") for c in range(JC)]
        for jc in range(JC):
            ones_cols = vp[jc][:].rearrange("p (h e) -> p h e", e=65)[:, :, 64:65]
            nc.gpsimd.memset(ones_cols, 1.0)
        # x^T, all qc chunks merged: col (qc, n) = qc*N + n
        xt_all = persist.tile([P, QC * N], BF16, name="xt_all")

        # ---------- psum pools (8 banks total) ----------
        spsum = est.enter_context(tc.tile_pool(name="spsum", bufs=2, space="PSUM"))
        pvps = est.enter_context(tc.tile_pool(name="pvps", bufs=2, space="PSUM"))
        gps = est.enter_context(tc.tile_pool(name="gps", bufs=2, space="PSUM"))

        # ---------- small staging pools ----------
        ring = est.enter_context(tc.tile_pool(name="ring", bufs=3))
        rdp = est.enter_context(tc.tile_pool(name="rdp", bufs=2))
        osp = est.enter_context(tc.tile_pool(name="osp", bufs=2))
        onp = est.enter_context(tc.tile_pool(name="onp", bufs=2))

        # ---------- weight pools ----------
        wq_pool = est.enter_context(tc.tile_pool(name="wq_pool", bufs=1))
        wq_sb = wq_pool.tile([P, QC * INNER], BF16, name="wq_sb")
        # pts must exist before the first S head (emitted before scope_a
        # closes), so it sits below scope_a on the allocator stack.
        ptsp = est.enter_context(tc.tile_pool(name="ptsp", bufs=2))

        scope_a = ExitStack()
        ctxt_p = scope_a.enter_context(tc.tile_pool(name="ctxt_p", bufs=1))
        wkv_p = scope_a.enter_context(tc.tile_pool(name="wkv_p", bufs=1))
        # ctx^T merged: col (cc, j) = cc*J + j
        ctxt_all = ctxt_p.tile([P, CC * J], BF16, name="ctxt_all")
        wk_sb = wkv_p.tile([P, CC * INNER], BF16, name="wk_sb")
        wv_sb = wkv_p.tile([P, CC * INNER], BF16, name="wv_sb")

        # ---------- front: loads + transposes + casts ----------
        def load_ctx(jc):
            cf = ring.tile([P, QD], F32, name="cf", tag="st")
            nc.sync.dma_start(cf[:, 0:CD], ctx_d[jc * P:(jc + 1) * P, :])
            for h3 in range(2):
                tp = gps.tile([P, NBW], F32, name="ctp", tag="gp")
                for c3 in range(3):
                    cc = h3 * 3 + c3
                    nc.tensor.transpose(
                        tp[:, c3 * P:(c3 + 1) * P],
                        cf[:, cc * P:(cc + 1) * P], ident32[:])
                dst = ctxt_all[:].rearrange("p (c j) -> p c j", c=CC)
                dst = dst[:, h3 * 3:(h3 + 1) * 3, jc * P:(jc + 1) * P]
                src = tp[:, 0:3 * P].rearrange("p (c j) -> p c j", c=3)
                # split psum evictions between DVE and ACT (both idle here)
                if jc % 2 == 0:
                    nc.vector.tensor_copy(dst, src)
                else:
                    nc.scalar.copy(dst, src)

        def load_wk_ic(ic):
            wf = ring.tile([P, QD], F32, name="wf", tag="st")
            nc.sync.dma_start(
                wf[:, 0:CC * P].rearrange("p (c n) -> p c n", c=CC),
                wk_d[:, ic * P:(ic + 1) * P].rearrange("(c p) n -> p c n", p=P))
            nc.gpsimd.tensor_copy(
                wk_sb[:].rearrange("p (c n) -> p c n", c=CC)[:, :, ic * P:(ic + 1) * P],
                wf[:, 0:CC * P].rearrange("p (c n) -> p c n", c=CC))

        def load_wq_ic(ic):
            wf = ring.tile([P, QD], F32, name="wf", tag="st")
            nc.sync.dma_start(
                wf[:].rearrange("p (c n) -> p c n", c=QC),
                wq_d[:, ic * P:(ic + 1) * P].rearrange("(c p) n -> p c n", p=P))
            nc.gpsimd.tensor_copy(
                wq_sb[:].rearrange("p (c n) -> p c n", c=QC)[:, :, ic * P:(ic + 1) * P],
                wf[:].rearrange("p (c n) -> p c n", c=QC))

        def load_wv_cc(cc):
            wf = ring.tile([P, QD], F32, name="wf", tag="st")
            nc.sync.dma_start(wf[:], wv_d[cc * P:(cc + 1) * P, :])
            nc.gpsimd.tensor_copy(
                wv_sb[:, cc * INNER:(cc + 1) * INNER], wf[:])

        def load_x_nt(nt):
            xf = ring.tile([P, QD], F32, name="xf", tag="st")
            nc.sync.dma_start(xf[:], x_d[nt * P:(nt + 1) * P, :])
            for qq in range(2):
                tp = gps.tile([P, NBW], F32, name="xtp", tag="gp")
                for q4 in range(4):
                    qc = qq * 4 + q4
                    nc.tensor.transpose(
                        tp[:, q4 * P:(q4 + 1) * P],
                        xf[:, qc * P:(qc + 1) * P], ident32[:])
                dst = xt_all[:].rearrange("p (c n) -> p c n", c=QC)
                dst = dst[:, qq * 4:(qq + 1) * 4, nt * P:(nt + 1) * P]
                nc.vector.tensor_copy(
                    dst, tp[:].rearrange("p (c n) -> p c n", c=4))

        # DMA order = priority: wq0, x half0, ctx, wk0 (everything gating
        # the first attention head), then wv, wq1/wk1, remaining weights.
        load_wq_ic(0)
        for nt in range(NTH):
            load_x_nt(nt)
        for jc in range(JC):
            load_ctx(jc)
        load_wk_ic(0)
        for cc in range(CC):
            load_wv_cc(cc)
        load_wq_ic(1)
        load_wk_ic(1)
        for ic in range(2, IC):
            load_wq_ic(ic)
            load_wk_ic(ic)

        # ---------- bias broadcast to 128 partitions ----------
        for qb in range(QD // NBW):
            bp = gps.tile([P, NBW], F32, name="bp", tag="gp")
            nc.tensor.matmul(
                bp[:], ones_row[:, :], bo_sb[:, qb * NBW:(qb + 1) * NBW],
                start=True, stop=True)
            nc.vector.tensor_copy(bias_bc[:, qb * NBW:(qb + 1) * NBW], bp[:])

        def emit_k(ic):
            for jb in range(J // NBW):
                kp = gps.tile([P, NBW], F32, name="kp", tag="gp")
                for cc in range(CC):
                    nc.tensor.matmul(
                        kp[:],
                        wk_sb[:, cc * INNER + ic * P: cc * INNER + (ic + 1) * P],
                        ctxt_all[:, cc * J + jb * NBW: cc * J + (jb + 1) * NBW],
                        start=(cc == 0), stop=(cc == CC - 1))
                nc.vector.tensor_copy(kt[ic][:, jb * NBW:(jb + 1) * NBW], kp[:])

        def emit_q(ic, n2):
            base = n2 * NH
            for nb in range(NH // NBW):
                qp = gps.tile([P, NBW], F32, name="qp", tag="gp")
                for qc in range(QC):
                    nc.tensor.matmul(
                        qp[:],
                        wq_sb[:, qc * INNER + ic * P: qc * INNER + (ic + 1) * P],
                        xt_all[:, qc * N + base + nb * NBW:
                               qc * N + base + (nb + 1) * NBW],
                        start=(qc == 0), stop=(qc == QC - 1))
                nc.vector.tensor_copy(
                    qt[ic][:, base + nb * NBW: base + (nb + 1) * NBW], qp[:])

        def emit_v(jc):
            for vb in range(INNER // NBW):
                vpp = gps.tile([P, NBW], F32, name="vpp", tag="gp")
                for cc in range(CC):
                    nc.tensor.matmul(
                        vpp[:],
                        ctxt_all[:, cc * J + jc * P: cc * J + (jc + 1) * P],
                        wv_sb[:, cc * INNER + vb * NBW: cc * INNER + (vb + 1) * NBW],
                        start=(cc == 0), stop=(cc == CC - 1))
                hpb = NBW // Dh  # 8 heads per 512 block
                dst = vp[jc][:, vb * hpb * 65:(vb + 1) * hpb * 65]
                dst = dst.rearrange("p (h e) -> p h e", e=65)[:, :, 0:64]
                src = vpp[:].rearrange("p (h e) -> p h e", e=Dh)
                nc.vector.tensor_copy(dst, src)

        # ---------- O^T aliases into xt_all ----------
        # The qc-chunk columns of a half are dead once Q-projection for
        # that half has consumed them, and ic ranges over the same 8
        # chunks: ot[n2][ic] = xt_all[:, ic*N + n2*NH ...].
        def ot(n2, ic):
            base = ic * N + n2 * NH
            return xt_all[:, base: base + NH]

        on_tiles = {}   # (h, quad) -> o_nat staging tile
        pts_cur = {}    # (h, n2) -> list of 8 pts tiles

        def emit_s_all(h, n2, pump=None):
            ic, po = h // 2, (h % 2) * Dh
            base = n2 * NH
            pts = []
            for jc in range(JC):
                sp = spsum.tile([P, NH], F32, name="sp", tag="sp")
                for nbh in range(NH // NBW):
                    nc.tensor.matmul(
                        sp[:, nbh * NBW:(nbh + 1) * NBW],
                        kt[ic][po:po + Dh, jc * P:(jc + 1) * P],
                        qt[ic][po:po + Dh, base + nbh * NBW: base + (nbh + 1) * NBW],
                        start=True, stop=True)
                pt = ptsp.tile([P, NH], BF16, name=f"pt{jc}", tag=f"pt{jc}")
                nc.scalar.activation(pt[:], sp[:], EXP, scale=SCALE)
                pts.append(pt)
                if pump is not None:
                    pump(jc)
            pts_cur[(h, n2)] = pts

        def emit_pv(h, n2):
            pts = pts_cur.pop((h, n2))
            for q in range(2):
                pvq = pvps.tile([P, 4 * P], F32, name="pv", tag="pv")
                for k in range(4):
                    nt_l = q * 4 + k
                    for jc in range(JC):
                        nc.tensor.matmul(
                            pvq[:, k * P: k * P + 65],
                            pts[jc][:, nt_l * P:(nt_l + 1) * P],
                            vp[jc][:, h * 65:(h + 1) * 65],
                            start=(jc == 0), stop=(jc == JC - 1))
                rd = rdp.tile([P, 4], F32, name="rd", tag="rd")
                den = pvq[:].rearrange("p (k c) -> p k c", c=P)[:, :, 64:65]
                nc.vector.reciprocal(rd[:].rearrange("p (k o) -> p k o", o=1), den)
                on = onp.tile([P, 4 * Dh], BF16, name="on", tag=f"on{h % 2}_{q}")
                for k in range(4):
                    nc.vector.tensor_scalar(
                        out=on[:, k * Dh:(k + 1) * Dh],
                        in0=pvq[:, k * P: k * P + Dh],
                        scalar1=rd[:, k:k + 1], scalar2=None, op0=MUL)
                on_tiles[(h, q)] = on

        def emit_t(pair, n2):
            for q in range(2):
                tp = gps.tile([P, NBW], BF16, name="tp", tag="gp")
                for par in range(2):
                    h = 2 * pair + par
                    po = par * Dh
                    on = on_tiles.pop((h, q))
                    for k in range(4):
                        nc.tensor.transpose(
                            tp[po:po + Dh, k * P:(k + 1) * P],
                            on[:, k * Dh:(k + 1) * Dh],
                            ident[:])
                nc.vector.tensor_copy(
                    ot(n2, pair)[:, q * NBW:(q + 1) * NBW], tp[:])

        def emit_v_half(jc, vb):
            vpp = gps.tile([P, NBW], F32, name="vpp", tag="gp")
            for cc in range(CC):
                nc.tensor.matmul(
                    vpp[:],
                    ctxt_all[:, cc * J + jc * P: cc * J + (jc + 1) * P],
                    wv_sb[:, cc * INNER + vb * NBW: cc * INNER + (vb + 1) * NBW],
                    start=(cc == 0), stop=(cc == CC - 1))
            hpb = NBW // Dh  # 8 heads per 512 block
            dst = vp[jc][:, vb * hpb * 65:(vb + 1) * hpb * 65]
            dst = dst.rearrange("p (h e) -> p h e", e=65)[:, :, 0:64]
            src = vpp[:].rearrange("p (h e) -> p h e", e=Dh)
            nc.vector.tensor_copy(dst, src)

        class Pump:
            """Ordered filler work, emitted bit-by-bit between S chunks."""

            def __init__(self):
                self.items = []
                self.idx = 0

            def add(self, fn, min_h=0):
                self.items.append((fn, min_h))

            def step(self, h=10 ** 9):
                if self.idx < len(self.items):
                    fn, min_h = self.items[self.idx]
                    if min_h <= h:
                        fn()
                        self.idx += 1

            def drain(self, upto=None):
                end = len(self.items) if upto is None else min(upto, len(self.items))
                while self.idx < end:
                    self.items[self.idx][0]()
                    self.idx += 1

        # ---------- upfront: heads 0-1, V(vb=0) ----------
        emit_k(0)
        emit_q(0, 0)
        emit_s_all(0, 0)
        pump_a = Pump()
        for jc in range(JC):
            pump_a.add(lambda jc=jc: emit_v_half(jc, 0))
        emit_s_all(1, 0, lambda jc: pump_a.step() if jc % 2 == 1 else None)
        pump_a.drain()
        emit_pv(0, 0)
        emit_k(1)
        emit_q(1, 0)

        # ---------- half 0 heads 2-15 ----------
        # Fillers in deadline order; heads 0-7 only read the vb=0 block of
        # vp, so V(vb=1) may lag until PV(8).
        pump_b = Pump()
        for ic in (2, 3):
            pump_b.add(lambda ic=ic: emit_q(ic, 0))     # items 0-1
        for ic in (2, 3):
            pump_b.add(lambda ic=ic: emit_k(ic))        # items 2-3
        for ic in (4, 5, 6, 7):
            pump_b.add(lambda ic=ic: emit_q(ic, 0))     # items 4-7
        for ic in (4, 5, 6, 7):
            pump_b.add(lambda ic=ic: emit_k(ic))        # items 8-11
        for jc in range(JC):
            pump_b.add(lambda jc=jc: emit_v_half(jc, 1))  # items 12-19
        for nt in range(NTH, NT):
            pump_b.add(lambda nt=nt: load_x_nt(nt))     # items 20-27
        for ic in (1, 2, 3):
            pump_b.add(lambda ic=ic: emit_q(ic, 1))     # items 28-30

        first_t = [True]

        def force_before_s(pump, h):
            # K/Q(h//2) and, for h >= 9, the V(vb=1) chains must be out.
            need = {4: 4, 5: 4, 6: 4, 7: 4, 8: 9, 9: 20, 10: 10,
                    11: 10, 12: 11, 13: 11, 14: 12, 15: 12}
            pump.drain(upto=need.get(h, 0))

        for h in range(2, H):
            force_before_s(pump_b, h)
            emit_s_all(
                h, 0, lambda jc, h=h: pump_b.step(h) if jc in (3, 7) else None)
            emit_pv(h - 1, 0)
            if h >= 3 and h % 2 == 1:
                if first_t[0]:
                    # T writes clobber the aliased xt_all half-0 columns:
                    # every half-0 Q projection must be emitted first.
                    pump_b.drain(upto=8)
                    first_t[0] = False
                emit_t((h - 3) // 2, 0)
        emit_pv(H - 1, 0)
        emit_t(H // 2 - 1, 0)
        pump_b.drain()

        # ctxT / wk / wv no longer referenced; reuse the space for Wo
        scope_a.close()
        wo_pool = est.enter_context(tc.tile_pool(name="wo_pool", bufs=1))
        wo_sb = wo_pool.tile([P, IC * QD], BF16, name="wo_sb")
        for ic in range(IC):
            wf = ring.tile([P, QD], F32, name="wf", tag="st")
            nc.sync.dma_start(wf[:], wo_d[ic * P:(ic + 1) * P, :])
            nc.gpsimd.tensor_copy(wo_sb[:, ic * QD:(ic + 1) * QD], wf[:])

        def emit_d(n2, chunk):
            nt_l, qb = chunk // 2, chunk % 2
            op = gps.tile([P, NBW], F32, name="op", tag="gp")
            for ic in range(IC):
                nc.tensor.matmul(
                    op[:],
                    ot(n2, ic)[:, nt_l * P:(nt_l + 1) * P],
                    wo_sb[:, ic * QD + qb * NBW: ic * QD + (qb + 1) * NBW],
                    start=(ic == 0), stop=(ic == IC - 1))
            os = osp.tile([P, NBW], F32, name="os", tag="os")
            nc.vector.tensor_tensor(
                os[:], op[:], bias_bc[:, qb * NBW:(qb + 1) * NBW], op=ADD)
            row = n2 * NH + nt_l * P
            nc.sync.dma_start(
                out_d[row:row + P, qb * NBW:(qb + 1) * NBW], os[:])

        # ---------- half 1: Q(half1), out-proj(half0), D1 pass-A woven ---
        emit_q(0, 1)
        # D(half 1) is split: pass A accumulates ic 0-3 (ready once T(3,1)
        # has run, h >= 10) into a bf16 staging tile with the bias; the
        # tail then only needs ic 4-7 per chunk.
        dstage = est.enter_context(tc.tile_pool(name="dstage", bufs=1))
        dst_t = [dstage.tile([P, NBW], BF16, name=f"dst{c}")
                 for c in range(2 * NTH)]

        def emit_d1a(chunk):
            nt_l, qb = chunk // 2, chunk % 2
            op = gps.tile([P, NBW], F32, name="op", tag="gp")
            for ic in range(IC // 2):
                nc.tensor.matmul(
                    op[:],
                    ot(1, ic)[:, nt_l * P:(nt_l + 1) * P],
                    wo_sb[:, ic * QD + qb * NBW: ic * QD + (qb + 1) * NBW],
                    start=(ic == 0), stop=(ic == IC // 2 - 1))
            nc.vector.tensor_tensor(
                dst_t[chunk][:], op[:], bias_bc[:, qb * NBW:(qb + 1) * NBW],
                op=ADD)

        pump_c = Pump()
        for ic in range(4, IC):
            pump_c.add(lambda ic=ic: emit_q(ic, 1))     # items 0-3
        for chunk in range(2 * NTH):
            pump_c.add(lambda chunk=chunk: emit_d(0, chunk))  # items 4-19
        for chunk in range(2 * NTH):
            pump_c.add(lambda chunk=chunk: emit_d1a(chunk), min_h=10)

        first_t1 = [True]
        for h in range(H):
            pump_c.drain(upto=max(0, h // 2 - 3))
            emit_s_all(
                h, 1, lambda jc, h=h: pump_c.step(h) if jc in (3, 7) else None)
            if h >= 1:
                emit_pv(h - 1, 1)
            if h >= 3 and h % 2 == 1:
                if first_t1[0]:
                    pump_c.drain(upto=4)   # all Q(half1) before first T
                    first_t1[0] = False
                emit_t((h - 3) // 2, 1)
        emit_pv(H - 1, 1)
        emit_t(H // 2 - 1, 1)
        pump_c.drain()

        # ---------- out-projection for half 1: pass B (ic 4-7 + stage) ----
        for chunk in range(2 * NTH):
            nt_l, qb = chunk // 2, chunk % 2
            pool = gps if chunk % 2 == 0 else pvps
            tag = "gp" if chunk % 2 == 0 else "pv"
            op = pool.tile([P, NBW], F32, name="op", tag=tag)
            for ic in range(IC // 2, IC):
                nc.tensor.matmul(
                    op[:],
                    ot(1, ic)[:, nt_l * P:(nt_l + 1) * P],
                    wo_sb[:, ic * QD + qb * NBW: ic * QD + (qb + 1) * NBW],
                    start=(ic == IC // 2), stop=(ic == IC - 1))
            os = osp.tile([P, NBW], F32, name="os", tag="os")
            nc.vector.tensor_tensor(os[:], op[:], dst_t[chunk][:], op=ADD)
            row = NH + nt_l * P
            nc.sync.dma_start(
                out_d[row:row + P, qb * NBW:(qb + 1) * NBW], os[:])


def _get_module():
    if "nc" not in _CACHE:
        _CACHE["nc"] = _build_module()
    return _CACHE["nc"]


def kernel(x, context, Wq, Wk, Wv, Wo, bo):
    nc = _get_module()
    x = np.asarray(x, dtype=np.float32)
    context = np.asarray(context, dtype=np.float32)
    Wq = np.asarray(Wq, dtype=np.float32)
    Wk = np.asarray(Wk, dtype=np.float32)
    Wv = np.asarray(Wv, dtype=np.float32)
    Wo = np.asarray(Wo, dtype=np.float32)
    bo = np.asarray(bo, dtype=np.float32)

    in_maps = [
        {
            "x": np.ascontiguousarray(x[b]),
            "context": np.ascontiguousarray(context[b]),
            "Wq": Wq, "Wk": Wk, "Wv": Wv, "Wo": Wo, "bo": bo,
        }
        for b in range(B)
    ]
    res = bass_utils.run_bass_kernel_spmd(nc, in_maps, core_ids=list(range(B)))
    return np.stack([res.results[b]["out"] for b in range(B)], axis=0)


if __name__ == "__main__":
    nc = _get_module()
    print("module built and compiled OK")


# revision 28
# speedup vs baseline: 1.4575x; 1.0525x over previous
"""Trainium2 Bass kernel for nn_CrossAttentionLayer (v3).

Reference computation (per batch element b):
    q = x @ Wq            [N, INNER]   (heads: INNER = H*Dh)
    k = ctx @ Wk          [J, INNER]
    v = ctx @ Wv          [J, INNER]
    sim = q_h @ k_h.T * scale   per head -> softmax over J -> @ v_h
    out = concat_heads @ Wo + bo

Sharding: batch (B=8) across 8 cores, one batch element per core, weights
replicated.  No collectives.

v3 design:
  - No DRAM->DRAM bf16 staging: f32 inputs stream through a small SBUF
    ring; ctx/x are transposed on the (otherwise idle) tensor engine at
    the front, weights are cast f32->bf16 on the (idle) gpsimd engine.
    Wq/Wk load per-ic so the first attention head starts ~40us in.
  - PV computed in natural-O orientation: stationary = P^T tile [128 j,
    128 n] (full contraction AND full output partitions), moving = V
    padded with a per-head ones column -> softmax denominator lands in
    PSUM column 64 of each slice for free.
  - Normalize with DVE reciprocal (batched, strided view) + tensor_scalar
    per-partition multiply; PE-transpose the normalized O tiles into O^T
    for the output projection.
  - N split in two halves: exp runs at [128, 1024] grain on ACT (its
    efficient grain) and the half-0 output projection overlaps half-1
    attention.  Q-projection for half 1, x-half-1 transposes and out-proj
    chunks are woven into the head loops so PE stays busy while ACT chews
    the exp stream.
"""

import sys

if "/opt/trn_rl_repo" not in sys.path:
    sys.path.insert(0, "/opt/trn_rl_repo")

from contextlib import ExitStack

import numpy as np

import concourse.bass as bass
import concourse.mybir as mybir
import concourse.bacc as bacc
import concourse.tile as tile
from concourse import bass_utils
from concourse.masks import make_identity

P = 128
B, N, J = 8, 2048, 1024
QD, CD, H, Dh = 1024, 768, 16, 64
INNER = H * Dh
NT = N // P      # 16 n tiles
JC = J // P      # 8 context chunks
QC = QD // P     # 8 x-feature chunks
CC = CD // P     # 6 ctx-feature chunks
IC = INNER // P  # 8 inner chunks
NH = N // 2      # 1024: half of the n dimension
NTH = NH // P    # 8 n tiles per half
NBW = 512        # psum matmul block width
SCALE = float(Dh) ** -0.5

F32 = mybir.dt.float32
BF16 = mybir.dt.bfloat16
F8 = mybir.dt.float8e4
DR = mybir.MatmulPerfMode.DoubleRow
EXP = mybir.ActivationFunctionType.Exp
MUL = mybir.AluOpType.mult
ADD = mybir.AluOpType.add

_CACHE = {}


def _build_module():
    nc = bacc.Bacc("TRN2", target_bir_lowering=False, debug=False)

    x_d = nc.dram_tensor("x", [N, QD], F32, kind="ExternalInput")
    ctx_d = nc.dram_tensor("context", [J, CD], F32, kind="ExternalInput")
    wq_d = nc.dram_tensor("Wq", [QD, INNER], F32, kind="ExternalInput")
    wk_d = nc.dram_tensor("Wk", [CD, INNER], F32, kind="ExternalInput")
    wv_d = nc.dram_tensor("Wv", [CD, INNER], F32, kind="ExternalInput")
    wo_d = nc.dram_tensor("Wo", [INNER, QD], F32, kind="ExternalInput")
    bo_d = nc.dram_tensor("bo", [QD], F32, kind="ExternalInput")
    out_d = nc.dram_tensor("out", [N, QD], F32, kind="ExternalOutput")

    with tile.TileContext(nc) as tc:
        _emit(nc, tc, x_d, ctx_d, wq_d, wk_d, wv_d, wo_d, bo_d, out_d)

    nc.compile()
    return nc


def _emit(nc, tc, x_d, ctx_d, wq_d, wk_d, wv_d, wo_d, bo_d, out_d):
    est = ExitStack()
    with est:
        # ---------- constants ----------
        const = est.enter_context(tc.tile_pool(name="const", bufs=1))
        ident = const.tile([P, P], BF16, name="ident")
        make_identity(nc, ident[:])
        ident32 = const.tile([P, P], F32, name="ident32")
        make_identity(nc, ident32[:])
        ones_row = const.tile([1, P], BF16, name="ones_row")
        nc.vector.memset(ones_row[:], 1.0)
        bo_sb = const.tile([1, QD], BF16, name="bo_sb")
        nc.gpsimd.dma_start(bo_sb[:], bo_d[:].unsqueeze(0))
        bias_bc = const.tile([P, QD], BF16, name="bias_bc")

        # ---------- persistent activations ----------
        persist = est.enter_context(tc.tile_pool(name="persist", bufs=1))
        # q^T hi/lo fp8 planes (cols 0:N and N:2N); k^T plain fp8
        qt = [persist.tile([P, 2 * N], F8, name=f"qt{c}") for c in range(IC)]
        kt = [persist.tile([P, J], F8, name=f"kt{c}") for c in range(IC)]
        # v natural per jc, padded per head: 64 v columns + a ones column
        vp = [persist.tile([P, H * 65], BF16, name=f"vp{c}") for c in range(JC)]
        for jc in range(JC):
            ones_cols = vp[jc][:].rearrange("p (h e) -> p h e", e=65)[:, :, 64:65]
            nc.gpsimd.memset(ones_cols, 1.0)
        # x^T, all qc chunks merged: col (qc, n) = qc*N + n
        xt_all = persist.tile([P, QC * N], BF16, name="xt_all")

        # ---------- psum pools (8 banks total) ----------
        spsum = est.enter_context(tc.tile_pool(name="spsum", bufs=2, space="PSUM"))
        pvps = est.enter_context(tc.tile_pool(name="pvps", bufs=2, space="PSUM"))
        gps = est.enter_context(tc.tile_pool(name="gps", bufs=2, space="PSUM"))

        # ---------- small staging pools ----------
        ring = est.enter_context(tc.tile_pool(name="ring", bufs=3))
        rdp = est.enter_context(tc.tile_pool(name="rdp", bufs=2))
        osp = est.enter_context(tc.tile_pool(name="osp", bufs=2))
        onp = est.enter_context(tc.tile_pool(name="onp", bufs=2))

        # ---------- weight pools ----------
        wq_pool = est.enter_context(tc.tile_pool(name="wq_pool", bufs=1))
        wq_sb = wq_pool.tile([P, QC * INNER], BF16, name="wq_sb")
        # pts must exist before the first S head (emitted before scope_a
        # closes), so it sits below scope_a on the allocator stack.
        ptsp = est.enter_context(tc.tile_pool(name="ptsp", bufs=2))

        scope_a = ExitStack()
        ctxt_p = scope_a.enter_context(tc.tile_pool(name="ctxt_p", bufs=1))
        wkv_p = scope_a.enter_context(tc.tile_pool(name="wkv_p", bufs=1))
        # ctx^T merged: col (cc, j) = cc*J + j
        ctxt_all = ctxt_p.tile([P, CC * J], BF16, name="ctxt_all")
        wk_sb = wkv_p.tile([P, CC * INNER], BF16, name="wk_sb")
        wv_sb = wkv_p.tile([P, CC * INNER], BF16, name="wv_sb")

        # ---------- front: loads + transposes + casts ----------
        def load_ctx(jc):
            cf = ring.tile([P, QD], F32, name="cf", tag="st")
            nc.sync.dma_start(cf[:, 0:CD], ctx_d[jc * P:(jc + 1) * P, :])
            for h3 in range(2):
                tp = gps.tile([P, NBW], F32, name="ctp", tag="gp")
                for c3 in range(3):
                    cc = h3 * 3 + c3
                    nc.tensor.transpose(
                        tp[:, c3 * P:(c3 + 1) * P],
                        cf[:, cc * P:(cc + 1) * P], ident32[:])
                dst = ctxt_all[:].rearrange("p (c j) -> p c j", c=CC)
                dst = dst[:, h3 * 3:(h3 + 1) * 3, jc * P:(jc + 1) * P]
                src = tp[:, 0:3 * P].rearrange("p (c j) -> p c j", c=3)
                # split psum evictions between DVE and ACT (both idle here)
                if jc % 2 == 0:
                    nc.vector.tensor_copy(dst, src)
                else:
                    nc.scalar.copy(dst, src)

        def load_wk_ic(ic):
            wf = ring.tile([P, QD], F32, name="wf", tag="st")
            nc.sync.dma_start(
                wf[:, 0:CC * P].rearrange("p (c n) -> p c n", c=CC),
                wk_d[:, ic * P:(ic + 1) * P].rearrange("(c p) n -> p c n", p=P))
            nc.gpsimd.tensor_copy(
                wk_sb[:].rearrange("p (c n) -> p c n", c=CC)[:, :, ic * P:(ic + 1) * P],
                wf[:, 0:CC * P].rearrange("p (c n) -> p c n", c=CC))

        def load_wq_ic(ic):
            wf = ring.tile([P, QD], F32, name="wf", tag="st")
            nc.sync.dma_start(
                wf[:].rearrange("p (c n) -> p c n", c=QC),
                wq_d[:, ic * P:(ic + 1) * P].rearrange("(c p) n -> p c n", p=P))
            nc.gpsimd.tensor_copy(
                wq_sb[:].rearrange("p (c n) -> p c n", c=QC)[:, :, ic * P:(ic + 1) * P],
                wf[:].rearrange("p (c n) -> p c n", c=QC))

        def load_wv_cc(cc):
            wf = ring.tile([P, QD], F32, name="wf", tag="st")
            nc.sync.dma_start(wf[:], wv_d[cc * P:(cc + 1) * P, :])
            nc.gpsimd.tensor_copy(
                wv_sb[:, cc * INNER:(cc + 1) * INNER], wf[:])

        def load_x_nt(nt):
            xf = ring.tile([P, QD], F32, name="xf", tag="st")
            nc.sync.dma_start(xf[:], x_d[nt * P:(nt + 1) * P, :])
            for qq in range(2):
                tp = gps.tile([P, NBW], F32, name="xtp", tag="gp")
                for q4 in range(4):
                    qc = qq * 4 + q4
                    nc.tensor.transpose(
                        tp[:, q4 * P:(q4 + 1) * P],
                        xf[:, qc * P:(qc + 1) * P], ident32[:])
                dst = xt_all[:].rearrange("p (c n) -> p c n", c=QC)
                dst = dst[:, qq * 4:(qq + 1) * 4, nt * P:(nt + 1) * P]
                nc.vector.tensor_copy(
                    dst, tp[:].rearrange("p (c n) -> p c n", c=4))

        # DMA order = priority: wq0, x half0, ctx, wk0 (everything gating
        # the first attention head), then wv, wq1/wk1, remaining weights.
        load_wq_ic(0)
        for nt in range(NTH):
            load_x_nt(nt)
        for jc in range(JC):
            load_ctx(jc)
        load_wk_ic(0)
        for cc in range(CC):
            load_wv_cc(cc)
        load_wq_ic(1)
        load_wk_ic(1)
        for ic in range(2, IC):
            load_wq_ic(ic)
            load_wk_ic(ic)

        # ---------- bias broadcast to 128 partitions ----------
        for qb in range(QD // NBW):
            bp = gps.tile([P, NBW], F32, name="bp", tag="gp")
            nc.tensor.matmul(
                bp[:], ones_row[:, :], bo_sb[:, qb * NBW:(qb + 1) * NBW],
                start=True, stop=True)
            nc.vector.tensor_copy(bias_bc[:, qb * NBW:(qb + 1) * NBW], bp[:])

        def emit_k(ic):
            for jb in range(J // NBW):
                kp = gps.tile([P, NBW], F32, name="kp", tag="gp")
                for cc in range(CC):
                    nc.tensor.matmul(
                        kp[:],
                        wk_sb[:, cc * INNER + ic * P: cc * INNER + (ic + 1) * P],
                        ctxt_all[:, cc * J + jb * NBW: cc * J + (jb + 1) * NBW],
                        start=(cc == 0), stop=(cc == CC - 1))
                nc.vector.tensor_copy(kt[ic][:, jb * NBW:(jb + 1) * NBW], kp[:])

        def emit_q(ic, n2):
            base = n2 * NH
            for nb in range(NH // NBW):
                qp = gps.tile([P, NBW], F32, name="qp", tag="gp")
                for qc in range(QC):
                    nc.tensor.matmul(
                        qp[:],
                        wq_sb[:, qc * INNER + ic * P: qc * INNER + (ic + 1) * P],
                        xt_all[:, qc * N + base + nb * NBW:
                               qc * N + base + (nb + 1) * NBW],
                        start=(qc == 0), stop=(qc == QC - 1))
                hi = qt[ic][:, base + nb * NBW: base + (nb + 1) * NBW]
                nc.vector.tensor_copy(hi, qp[:])
                nc.vector.tensor_tensor(
                    qt[ic][:, N + base + nb * NBW: N + base + (nb + 1) * NBW],
                    qp[:], hi, op=mybir.AluOpType.subtract)

        def emit_v(jc):
            for vb in range(INNER // NBW):
                vpp = gps.tile([P, NBW], F32, name="vpp", tag="gp")
                for cc in range(CC):
                    nc.tensor.matmul(
                        vpp[:],
                        ctxt_all[:, cc * J + jc * P: cc * J + (jc + 1) * P],
                        wv_sb[:, cc * INNER + vb * NBW: cc * INNER + (vb + 1) * NBW],
                        start=(cc == 0), stop=(cc == CC - 1))
                hpb = NBW // Dh  # 8 heads per 512 block
                dst = vp[jc][:, vb * hpb * 65:(vb + 1) * hpb * 65]
                dst = dst.rearrange("p (h e) -> p h e", e=65)[:, :, 0:64]
                src = vpp[:].rearrange("p (h e) -> p h e", e=Dh)
                nc.vector.tensor_copy(dst, src)

        # ---------- O^T aliases into xt_all ----------
        # The qc-chunk columns of a half are dead once Q-projection for
        # that half has consumed them, and ic ranges over the same 8
        # chunks: ot[n2][ic] = xt_all[:, ic*N + n2*NH ...].
        def ot(n2, ic):
            base = ic * N + n2 * NH
            return xt_all[:, base: base + NH]

        on_tiles = {}   # (h, quad) -> o_nat staging tile
        pts_cur = {}    # (h, n2) -> list of 8 pts tiles

        def emit_s_all(h, n2, pump=None):
            ic, po = h // 2, (h % 2) * Dh
            base = n2 * NH
            pts = []
            for jc in range(JC):
                sp = spsum.tile([P, NH], F32, name="sp", tag="sp")
                lhsT = kt[ic][po:po + Dh, jc * P:(jc + 1) * P]
                lhsT = lhsT.unsqueeze(1).broadcast_to([Dh, 2, P])
                rhs_pl = qt[ic][po:po + Dh, :].rearrange(
                    "p (two n) -> p two n", two=2)
                for nbh in range(NH // NBW):
                    nc.tensor.matmul(
                        sp[:, nbh * NBW:(nbh + 1) * NBW],
                        lhsT,
                        rhs_pl[:, :, base + nbh * NBW: base + (nbh + 1) * NBW],
                        start=True, stop=True, perf_mode=DR)
                pt = ptsp.tile([P, NH], BF16, name=f"pt{jc}", tag=f"pt{jc}")
                nc.scalar.activation(pt[:], sp[:], EXP, scale=SCALE)
                pts.append(pt)
                if pump is not None:
                    pump(jc)
            pts_cur[(h, n2)] = pts

        def emit_pv(h, n2):
            pts = pts_cur.pop((h, n2))
            for q in range(2):
                pvq = pvps.tile([P, 4 * P], F32, name="pv", tag="pv")
                for k in range(4):
                    nt_l = q * 4 + k
                    for jc in range(JC):
                        nc.tensor.matmul(
                            pvq[:, k * P: k * P + 65],
                            pts[jc][:, nt_l * P:(nt_l + 1) * P],
                            vp[jc][:, h * 65:(h + 1) * 65],
                            start=(jc == 0), stop=(jc == JC - 1))
                rd = rdp.tile([P, 4], F32, name="rd", tag="rd")
                den = pvq[:].rearrange("p (k c) -> p k c", c=P)[:, :, 64:65]
                nc.vector.reciprocal(rd[:].rearrange("p (k o) -> p k o", o=1), den)
                on = onp.tile([P, 4 * Dh], BF16, name="on", tag=f"on{h % 2}_{q}")
                for k in range(4):
                    nc.vector.tensor_scalar(
                        out=on[:, k * Dh:(k + 1) * Dh],
                        in0=pvq[:, k * P: k * P + Dh],
                        scalar1=rd[:, k:k + 1], scalar2=None, op0=MUL)
                on_tiles[(h, q)] = on

        def emit_t(pair, n2):
            for q in range(2):
                tp = gps.tile([P, NBW], BF16, name="tp", tag="gp")
                for par in range(2):
                    h = 2 * pair + par
                    po = par * Dh
                    on = on_tiles.pop((h, q))
                    for k in range(4):
                        nc.tensor.transpose(
                            tp[po:po + Dh, k * P:(k + 1) * P],
                            on[:, k * Dh:(k + 1) * Dh],
                            ident[:])
                nc.vector.tensor_copy(
                    ot(n2, pair)[:, q * NBW:(q + 1) * NBW], tp[:])

        def emit_v_half(jc, vb):
            vpp = gps.tile([P, NBW], F32, name="vpp", tag="gp")
            for cc in range(CC):
                nc.tensor.matmul(
                    vpp[:],
                    ctxt_all[:, cc * J + jc * P: cc * J + (jc + 1) * P],
                    wv_sb[:, cc * INNER + vb * NBW: cc * INNER + (vb + 1) * NBW],
                    start=(cc == 0), stop=(cc == CC - 1))
            hpb = NBW // Dh  # 8 heads per 512 block
            dst = vp[jc][:, vb * hpb * 65:(vb + 1) * hpb * 65]
            dst = dst.rearrange("p (h e) -> p h e", e=65)[:, :, 0:64]
            src = vpp[:].rearrange("p (h e) -> p h e", e=Dh)
            nc.vector.tensor_copy(dst, src)

        class Pump:
            """Ordered filler work, emitted bit-by-bit between S chunks."""

            def __init__(self):
                self.items = []
                self.idx = 0

            def add(self, fn, min_h=0):
                self.items.append((fn, min_h))

            def step(self, h=10 ** 9):
                if self.idx < len(self.items):
                    fn, min_h = self.items[self.idx]
                    if min_h <= h:
                        fn()
                        self.idx += 1

            def drain(self, upto=None):
                end = len(self.items) if upto is None else min(upto, len(self.items))
                while self.idx < end:
                    self.items[self.idx][0]()
                    self.idx += 1

        # ---------- upfront: heads 0-1, V(vb=0) ----------
        emit_k(0)
        emit_q(0, 0)
        emit_s_all(0, 0)
        pump_a = Pump()
        for jc in range(JC):
            pump_a.add(lambda jc=jc: emit_v_half(jc, 0))
        emit_s_all(1, 0, lambda jc: pump_a.step() if jc % 2 == 1 else None)
        pump_a.drain()
        emit_pv(0, 0)
        emit_k(1)
        emit_q(1, 0)

        # ---------- half 0 heads 2-15 ----------
        # Fillers in deadline order; heads 0-7 only read the vb=0 block of
        # vp, so V(vb=1) may lag until PV(8).
        pump_b = Pump()
        for ic in (2, 3):
            pump_b.add(lambda ic=ic: emit_q(ic, 0))     # items 0-1
        for ic in (2, 3):
            pump_b.add(lambda ic=ic: emit_k(ic))        # items 2-3
        for ic in (4, 5, 6, 7):
            pump_b.add(lambda ic=ic: emit_q(ic, 0))     # items 4-7
        for ic in (4, 5, 6, 7):
            pump_b.add(lambda ic=ic: emit_k(ic))        # items 8-11
        for jc in range(JC):
            pump_b.add(lambda jc=jc: emit_v_half(jc, 1))  # items 12-19
        for nt in range(NTH, NT):
            pump_b.add(lambda nt=nt: load_x_nt(nt))     # items 20-27
        for ic in (1, 2, 3):
            pump_b.add(lambda ic=ic: emit_q(ic, 1))     # items 28-30

        first_t = [True]

        def force_before_s(pump, h):
            # K/Q(h//2) and, for h >= 9, the V(vb=1) chains must be out.
            need = {4: 4, 5: 4, 6: 4, 7: 4, 8: 9, 9: 20, 10: 10,
                    11: 10, 12: 11, 13: 11, 14: 12, 15: 12}
            pump.drain(upto=need.get(h, 0))

        for h in range(2, H):
            force_before_s(pump_b, h)
            emit_s_all(
                h, 0, lambda jc, h=h: pump_b.step(h) if jc in (3, 7) else None)
            emit_pv(h - 1, 0)
            if h >= 3 and h % 2 == 1:
                if first_t[0]:
                    # T writes clobber the aliased xt_all half-0 columns:
                    # every half-0 Q projection must be emitted first.
                    pump_b.drain(upto=8)
                    first_t[0] = False
                emit_t((h - 3) // 2, 0)
        emit_pv(H - 1, 0)
        emit_t(H // 2 - 1, 0)
        pump_b.drain()

        # ctxT / wk / wv no longer referenced; reuse the space for Wo
        scope_a.close()
        wo_pool = est.enter_context(tc.tile_pool(name="wo_pool", bufs=1))
        wo_sb = wo_pool.tile([P, IC * QD], BF16, name="wo_sb")
        for ic in range(IC):
            wf = ring.tile([P, QD], F32, name="wf", tag="st")
            nc.sync.dma_start(wf[:], wo_d[ic * P:(ic + 1) * P, :])
            nc.gpsimd.tensor_copy(wo_sb[:, ic * QD:(ic + 1) * QD], wf[:])

        def emit_d(n2, chunk):
            nt_l, qb = chunk // 2, chunk % 2
            op = gps.tile([P, NBW], F32, name="op", tag="gp")
            for ic in range(IC):
                nc.tensor.matmul(
                    op[:],
                    ot(n2, ic)[:, nt_l * P:(nt_l + 1) * P],
                    wo_sb[:, ic * QD + qb * NBW: ic * QD + (qb + 1) * NBW],
                    start=(ic == 0), stop=(ic == IC - 1))
            os = osp.tile([P, NBW], F32, name="os", tag="os")
            nc.vector.tensor_tensor(
                os[:], op[:], bias_bc[:, qb * NBW:(qb + 1) * NBW], op=ADD)
            row = n2 * NH + nt_l * P
            nc.sync.dma_start(
                out_d[row:row + P, qb * NBW:(qb + 1) * NBW], os[:])

        # ---------- half 1: Q(half1), out-proj(half0), D1 pass-A woven ---
        emit_q(0, 1)
        # D(half 1) is split: pass A accumulates ic 0-3 (ready once T(3,1)
        # has run, h >= 10) into a bf16 staging tile with the bias; the
        # tail then only needs ic 4-7 per chunk.
        dstage = est.enter_context(tc.tile_pool(name="dstage", bufs=1))
        dst_t = [dstage.tile([P, NBW], BF16, name=f"dst{c}")
                 for c in range(2 * NTH)]

        def emit_d1a(chunk):
            nt_l, qb = chunk // 2, chunk % 2
            op = gps.tile([P, NBW], F32, name="op", tag="gp")
            for ic in range(IC // 2):
                nc.tensor.matmul(
                    op[:],
                    ot(1, ic)[:, nt_l * P:(nt_l + 1) * P],
                    wo_sb[:, ic * QD + qb * NBW: ic * QD + (qb + 1) * NBW],
                    start=(ic == 0), stop=(ic == IC // 2 - 1))
            nc.vector.tensor_tensor(
                dst_t[chunk][:], op[:], bias_bc[:, qb * NBW:(qb + 1) * NBW],
                op=ADD)

        pump_c = Pump()
        for ic in range(4, IC):
            pump_c.add(lambda ic=ic: emit_q(ic, 1))     # items 0-3
        for chunk in range(2 * NTH):
            pump_c.add(lambda chunk=chunk: emit_d(0, chunk))  # items 4-19
        for chunk in range(2 * NTH):
            pump_c.add(lambda chunk=chunk: emit_d1a(chunk), min_h=10)

        first_t1 = [True]
        for h in range(H):
            pump_c.drain(upto=max(0, h // 2 - 3))
            emit_s_all(
                h, 1, lambda jc, h=h: pump_c.step(h) if jc in (3, 7) else None)
            if h >= 1:
                emit_pv(h - 1, 1)
            if h >= 3 and h % 2 == 1:
                if first_t1[0]:
                    pump_c.drain(upto=4)   # all Q(half1) before first T
                    first_t1[0] = False
                emit_t((h - 3) // 2, 1)
        emit_pv(H - 1, 1)
        emit_t(H // 2 - 1, 1)
        pump_c.drain()

        # ---------- out-projection for half 1: pass B (ic 4-7 + stage) ----
        for chunk in range(2 * NTH):
            nt_l, qb = chunk // 2, chunk % 2
            pool = gps if chunk % 2 == 0 else pvps
            tag = "gp" if chunk % 2 == 0 else "pv"
            op = pool.tile([P, NBW], F32, name="op", tag=tag)
            for ic in range(IC // 2, IC):
                nc.tensor.matmul(
                    op[:],
                    ot(1, ic)[:, nt_l * P:(nt_l + 1) * P],
                    wo_sb[:, ic * QD + qb * NBW: ic * QD + (qb + 1) * NBW],
                    start=(ic == IC // 2), stop=(ic == IC - 1))
            os = osp.tile([P, NBW], F32, name="os", tag="os")
            nc.vector.tensor_tensor(os[:], op[:], dst_t[chunk][:], op=ADD)
            row = NH + nt_l * P
            nc.sync.dma_start(
                out_d[row:row + P, qb * NBW:(qb + 1) * NBW], os[:])


def _get_module():
    if "nc" not in _CACHE:
        _CACHE["nc"] = _build_module()
    return _CACHE["nc"]


def kernel(x, context, Wq, Wk, Wv, Wo, bo):
    nc = _get_module()
    x = np.asarray(x, dtype=np.float32)
    context = np.asarray(context, dtype=np.float32)
    Wq = np.asarray(Wq, dtype=np.float32)
    Wk = np.asarray(Wk, dtype=np.float32)
    Wv = np.asarray(Wv, dtype=np.float32)
    Wo = np.asarray(Wo, dtype=np.float32)
    bo = np.asarray(bo, dtype=np.float32)

    in_maps = [
        {
            "x": np.ascontiguousarray(x[b]),
            "context": np.ascontiguousarray(context[b]),
            "Wq": Wq, "Wk": Wk, "Wv": Wv, "Wo": Wo, "bo": bo,
        }
        for b in range(B)
    ]
    res = bass_utils.run_bass_kernel_spmd(nc, in_maps, core_ids=list(range(B)))
    return np.stack([res.results[b]["out"] for b in range(B)], axis=0)


if __name__ == "__main__":
    nc = _get_module()
    print("module built and compiled OK")
